# revision 14
# baseline (speedup 1.0000x reference)
"""GNN message passing (SpMM) on 8 Trainium2 NeuronCores.

Computes out = segment_sum((X @ W)[cols] * vals, rows) for
X [50000, 128] f32, W [128, 128], 800k edges -- as
out = segment_sum(vals * X[cols]) @ W  (linearity), so the device
gathers raw X rows (bf16 table), accumulates per-destination via
one-hot matmul-scatter on the TensorEngine, and applies W once per
128-destination block.

Sharding: destinations split evenly across the 8 cores (6250 each);
edges partitioned by destination. Each core's edges are grouped by
(dest-block of 128, source-window of 25000) and sorted by source so
gather indices fit in int16 (dma_gather requirement). One SPMD
program serves all 8 cores: per-core blocks are rank-matched by edge
count to program "slots" so the per-slot tile counts (max across
cores) stay close to each core's actual counts; pad edges carry
val=0. The host un-permutes output blocks.
"""

import numpy as np
import ml_dtypes

import concourse.bacc as bacc
import concourse.bass as bass
import concourse.mybir as mybir
import concourse.tile as tile
from concourse.bass_utils import run_bass_kernel_spmd

N_NODES = 50000
N_EDGES = 800000
F = 128
NCORES = 8
NPC = N_NODES // NCORES          # 6250 destinations per core
BLK = 128
NB = (NPC + BLK - 1) // BLK      # 49 blocks (last has 106 rows)
NWIN = 2
WINBASE = N_NODES // NWIN        # 25000 (< int16 max 32767)
CB = 1                           # block-slots per gather call
CHUNKS = [list(range(s, min(s + CB, NB))) for s in range(0, NB, CB)]
NCHUNK = len(CHUNKS)

BF16 = mybir.dt.bfloat16
F32 = mybir.dt.float32
I16 = mybir.dt.int16

_CACHE = {}


def _prep(adj_rows, adj_cols, adj_vals):
    """Host-side sharding: returns (structure, per-core arrays, block perm)."""
    dst = np.asarray(adj_rows).astype(np.int64)
    src = np.asarray(adj_cols).astype(np.int64)
    val = np.asarray(adj_vals).astype(np.float32)

    core = dst // NPC
    block = (dst % NPC) // BLK
    drel = (dst % NPC) % BLK
    win = (src >= WINBASE).astype(np.int64)

    key = (core * NB + block) * NWIN + win
    cnt = np.bincount(key, minlength=NCORES * NB * NWIN).reshape(NCORES, NB, NWIN)

    # rank-match blocks across cores: slot s = core k's rank-s block (by
    # total edge count) so max_k cnt stays close to each core's count
    perm = np.argsort(-(cnt.sum(axis=2)), axis=1, kind="stable")  # [NCORES, NB]
    slot_of_block = np.empty_like(perm)
    for k in range(NCORES):
        slot_of_block[k, perm[k]] = np.arange(NB)
    cnt_slot = np.take_along_axis(cnt, perm[:, :, None], axis=1)  # [NCORES, S, W]

    t = ((cnt_slot + BLK - 1) // BLK).max(axis=0)  # [NB(slots), NWIN]
    t[:, 0] = np.maximum(t[:, 0], 1)

    # call structure: one slot per call; fetch only the exact (16-aligned)
    # max-over-cores token count -- the tile tail beyond it stays unfetched
    # (buffers are memset once at startup so stale bytes are valid floats).
    calloff = np.zeros((NB, NWIN), dtype=np.int64)
    ncall = np.zeros((NCHUNK, NWIN), dtype=np.int64)
    mx = cnt_slot.max(axis=0)  # [NB, NWIN]
    for c, blocks in enumerate(CHUNKS):
        for w in range(NWIN):
            o = 0
            for s in blocks:
                calloff[s, w] = o
                o += t[s, w]
            m = int(mx[blocks[0], w]) if len(blocks) == 1 else None
            assert m is not None, "exact fetch needs single-slot chunks"
            if w == 0:
                m = max(m, 1)
            ncall[c, w] = (m + 15) // 16 * 16 if m > 0 else 0
    idx_off = np.zeros((NCHUNK, NWIN), dtype=np.int64)
    o = 0
    for c in range(NCHUNK):
        for w in range(NWIN):
            idx_off[c, w] = o
            o += ncall[c, w] // 16
    idx_cols = int(o)

    slot_base = np.concatenate([[0], np.cumsum(t[:, 0] + t[:, 1])[:-1]])
    nt = int((t[:, 0] + t[:, 1]).sum())

    # per-edge placement
    order = np.lexsort((src, key))
    key_s = key[order]
    gstart = np.zeros(NCORES * NB * NWIN + 1, dtype=np.int64)
    np.cumsum(cnt.ravel(), out=gstart[1:])
    rank = np.arange(N_EDGES, dtype=np.int64) - gstart[key_s]

    core_s = core[order]
    block_s = block[order]
    win_s = win[order]
    drel_s = drel[order]
    src_s = src[order]
    val_s = val[order]
    slot_s = slot_of_block[core_s, block_s]
    chunk_s = slot_s // CB
    j_s = rank // BLK
    p_s = rank % BLK
    dv_col = slot_base[slot_s] + np.where(win_s == 0, j_s, t[slot_s, 0] + j_s)
    q = BLK * (calloff[slot_s, win_s] + j_s) + p_s
    icol = idx_off[chunk_s, win_s] + q // 16
    irow = q % 16

    idxbase = np.zeros((NCORES, 16, idx_cols), dtype=np.int16)
    idxbase[core_s, irow, icol] = (src_s - win_s * WINBASE).astype(np.int16)
    dmat = np.zeros((NCORES, BLK, nt), dtype=np.float32)
    dmat[core_s, p_s, dv_col] = drel_s.astype(np.float32)
    vmat = np.zeros((NCORES, BLK, nt), dtype=np.float32)
    vmat[core_s, p_s, dv_col] = val_s

    idx = np.tile(idxbase, (1, 8, 1))

    struct = dict(
        t=t, calloff=calloff, ncall=ncall, idx_off=idx_off,
        idx_cols=idx_cols, slot_base=slot_base, nt=nt,
    )
    return struct, idx, dmat, vmat, perm


def _build(struct, rep=1, gbufs=6):
    t = struct["t"]
    calloff = struct["calloff"]
    ncall = struct["ncall"]
    idx_off = struct["idx_off"]
    slot_base = struct["slot_base"]
    nt = struct["nt"]

    nc = bacc.Bacc("TRN2", debug=False, num_swdge_queues=4)
    x = nc.declare_dram_parameter("x", [N_NODES, F], BF16, isOutput=False)
    wm = nc.declare_dram_parameter("wm", [F, F], BF16, isOutput=False)
    iotam = nc.declare_dram_parameter("iotam", [BLK, BLK], BF16, isOutput=False)
    idxp = nc.declare_dram_parameter(
        "idx", [BLK, struct["idx_cols"]], I16, isOutput=False
    )
    dmatp = nc.declare_dram_parameter("dmat", [BLK, nt], F32, isOutput=False)
    vmatp = nc.declare_dram_parameter("vmat", [BLK, nt], F32, isOutput=False)
    outp = nc.declare_dram_parameter("out", [NB * BLK, F], BF16, isOutput=True)

    xw = [x[0:WINBASE, :], x[WINBASE:N_NODES, :]]

    with tile.TileContext(nc) as tc:
        with (
            tc.tile_pool(name="const", bufs=1) as constp,
            tc.tile_pool(name="g0", bufs=gbufs) as g0p,
            tc.tile_pool(name="g1", bufs=gbufs) as g1p,
            tc.tile_pool(name="st", bufs=6) as stp,
            tc.tile_pool(name="psa", bufs=3, space="PSUM") as psap,
            tc.tile_pool(name="pso", bufs=2, space="PSUM") as psop,
            tc.tile_pool(name="acct", bufs=3) as acctp,
            tc.tile_pool(name="outs", bufs=1) as outsp,
        ):
            d_t = constp.tile([BLK, nt], F32, tag="d_t")
            nc.sync.dma_start(out=d_t[:], in_=dmatp[:])
            v_t = constp.tile([BLK, nt], F32, tag="v_t")
            nc.sync.dma_start(out=v_t[:], in_=vmatp[:])
            w_t = constp.tile([F, F], BF16, tag="w_t")
            nc.sync.dma_start(out=w_t[:], in_=wm[:])
            iota_t = constp.tile([BLK, BLK], BF16, tag="iota_t")
            nc.sync.dma_start(out=iota_t[:], in_=iotam[:])
            out_stage = outsp.tile([BLK, NB, F], BF16)

            # warm every gather buffer once so tile tails beyond the
            # exact fetch count hold valid floats (NaN x 0 would poison PSUM)
            for w, pool in ((0, g0p), (1, g1p)):
                mt = int(t[:, w].max())
                if mt == 0:
                    continue
                for _ in range(gbufs):
                    wt = pool.tile([BLK, mt, F], BF16, tag=f"g{w}")
                    nc.vector.memset(wt[:], 0)

            # per-call idx tiles so early gathers don't wait on one big load
            idx_tiles = {}
            for c in range(NCHUNK):
                for w in range(NWIN):
                    n = int(ncall[c, w])
                    if n == 0:
                        continue
                    io = int(idx_off[c, w])
                    it = constp.tile([BLK, n // 16], I16, tag=f"idx_{c}_{w}")
                    nc.sync.dma_start(out=it[:], in_=idxp[:, io : io + n // 16])
                    idx_tiles[(c, w)] = it

            import contextlib

            loop_ctx = (
                tc.For_i(0, rep, 1) if rep > 1 else contextlib.nullcontext()
            )
            with loop_ctx:
                qn = 0
                for c, blocks in enumerate(CHUNKS):
                    g = []
                    for w in range(NWIN):
                        n = int(ncall[c, w])
                        if n == 0:
                            g.append(None)
                            continue
                        T = sum(int(t[s, w]) for s in blocks)
                        gt = (g0p if w == 0 else g1p).tile(
                            [BLK, T, F], BF16, tag=f"g{w}"
                        )
                        nc.gpsimd.dma_gather(
                            gt[:], xw[w], idx_tiles[(c, w)][:], n, n, F,
                            single_packet=False, queue_num=qn % 4,
                        )
                        qn += 1
                        g.append(gt)
                    for s in blocks:
                        ntile_b = int(t[s, 0] + t[s, 1])
                        acc = psap.tile([F, BLK], F32, tag="acc")
                        k = 0
                        for w in range(NWIN):
                            for j in range(int(t[s, w])):
                                st = stp.tile([BLK, BLK], BF16, tag="st")
                                col = int(
                                    slot_base[s] + (j if w == 0 else t[s, 0] + j)
                                )
                                nc.vector.tensor_scalar(
                                    out=st[:],
                                    in0=iota_t[:],
                                    scalar1=d_t[:, col : col + 1],
                                    scalar2=v_t[:, col : col + 1],
                                    op0=mybir.AluOpType.is_equal,
                                    op1=mybir.AluOpType.mult,
                                )
                                gtile = g[w][:, int(calloff[s, w] + j), :]
                                nc.tensor.matmul(
                                    out=acc[:],
                                    lhsT=gtile,
                                    rhs=st[:],
                                    start=(k == 0),
                                    stop=(k == ntile_b - 1),
                                )
                                k += 1
                        acct = acctp.tile([F, BLK], BF16, tag="acct")
                        nc.scalar.copy(out=acct[:], in_=acc[:])
                        ops = psop.tile([BLK, F], F32, tag="ops")
                        nc.tensor.matmul(
                            out=ops[:], lhsT=acct[:], rhs=w_t[:], start=True,
                            stop=True,
                        )
                        nc.scalar.copy(out=out_stage[:, s, :], in_=ops[:])
                    # stream this chunk's output slice out now
                    lo, hi = blocks[0], blocks[-1] + 1
                    out_ap = outp[lo * BLK : hi * BLK, :].rearrange(
                        "(b d) o -> d b o", d=BLK
                    )
                    nc.sync.dma_start(out=out_ap, in_=out_stage[:, lo:hi, :])
    nc.compile()
    return nc


_LAST_STRUCT = None


def _in_maps_for(inputs, idx, dmat, vmat):
    xb = np.asarray(inputs["input"], dtype=np.float32).astype(ml_dtypes.bfloat16)
    wb = np.asarray(inputs["weight"], dtype=np.float32).astype(ml_dtypes.bfloat16)
    iota = np.tile(np.arange(BLK, dtype=np.float32), (BLK, 1)).astype(
        ml_dtypes.bfloat16
    )
    return [
        {"x": xb, "wm": wb, "iotam": iota, "idx": idx[k],
         "dmat": dmat[k], "vmat": vmat[k]}
        for k in range(NCORES)
    ]


def _timing_handles(inputs):
    """(nc_rep1, in_maps) for the rep-delta timing harness in test.py."""
    global _LAST_STRUCT
    struct, idx, dmat, vmat, perm = _prep(
        inputs["adj_rows"], inputs["adj_cols"], inputs["adj_vals"]
    )
    _LAST_STRUCT = struct
    ckey = (struct["idx_cols"], struct["nt"], struct["t"].tobytes(),
            struct["ncall"].tobytes())
    if ckey not in _CACHE:
        _CACHE[ckey] = _build(struct)
    return _CACHE[ckey], _in_maps_for(inputs, idx, dmat, vmat)


def _build_rep(R):
    assert _LAST_STRUCT is not None
    return _build(_LAST_STRUCT, rep=R)


def kernel(input, weight, adj_rows, adj_cols, adj_vals):
    x = np.asarray(input, dtype=np.float32)
    w = np.asarray(weight, dtype=np.float32)

    struct, idx, dmat, vmat, perm = _prep(adj_rows, adj_cols, adj_vals)

    ckey = (struct["idx_cols"], struct["nt"], struct["t"].tobytes(),
            struct["ncall"].tobytes())
    if ckey in _CACHE:
        nc = _CACHE[ckey]
    else:
        nc = _build(struct)
        _CACHE[ckey] = nc

    in_maps = _in_maps_for(
        {"input": x, "weight": w}, idx, dmat, vmat
    )
    res = run_bass_kernel_spmd(nc, in_maps, core_ids=list(range(NCORES)))

    out = np.empty((N_NODES, F), dtype=np.float32)
    for k in range(NCORES):
        r = np.asarray(res.results[k]["out"]).astype(np.float32).reshape(
            NB, BLK, F
        )
        for s in range(NB):
            b = perm[k, s]
            n = min(BLK, NPC - b * BLK)
            out[k * NPC + b * BLK : k * NPC + b * BLK + n] = r[s, :n]
    return out

